# revision 7
# baseline (speedup 1.0000x reference)
"""Trainium2 Bass kernel for nn_MixMLP (moe_routing) — fp8 DoubleRow version.

Strategy:
  - Output is binary: y_hard + y_soft - stop_grad(y_soft) == y_hard numerically,
    so each edge decision is  (logit0 - logit1) + (gum0 - gum1) >= 0.
  - Only the DIFFERENCE of adjacent final-layer columns matters:
        d = h3 @ wd,  wd = w3[:, 0::2] - w3[:, 1::2]   (1024 x 32640)
    decision = (d + bdd - gd) >= 0,  gd = gum1 - gum0, bdd = b3[0::2]-b3[1::2].
  - Rows are routed to one expert by mask = x[:,0] > 0. Host sorts rows so
    row-chunks of 128 are single-expert; 8 cores = 2 row-groups (one per
    expert) x 4 column-quarters of wd.
  - Device computes ONLY d (scaled): small MLP in bf16, h3 emitted as
    fp8e4m3 (scale BETA), wd pre-scaled by ALPHA to fp8e4m3; big matmul
    runs in DoubleRow mode (256-contraction per instruction, 2x fp8 rate).
    gumbel never touches the device; host subtracts gd exactly.
  - Host: dec = (t >= 0) with t = d + bdd - gd; near-ties |t| < tol are
    recomputed exactly in float64, then scattered into the symmetric
    adjacency output. Result is exact regardless of device matmul precision.
"""

import os
import numpy as np
import ml_dtypes

import concourse.bass as bass
import concourse.mybir as mybir
import concourse.tile as tile
from concourse import bacc
from concourse.bass_utils import run_bass_kernel_spmd

B = 512
COND = 64
N_NODES = 256
E = 32640  # upper-tri edges
NCORES = 8
QCOLS = E // 4  # 8160 columns of wd per core
QP = 8192  # padded to multiple of 1024
ARCH = [256, 512, 1024]

ALPHA = 512.0  # wd scale into fp8
BETA = 32.0  # h3 scale into fp8
SCALE = ALPHA * BETA

F32 = mybir.dt.float32
BF16 = mybir.dt.bfloat16
FP8 = mybir.dt.float8e4
NP_FP8 = ml_dtypes.float8_e4m3
NP_BF16 = ml_dtypes.bfloat16

# |t| < TOL_ABS + TOL_REL*|d| edges are recomputed exactly on host
TOL_ABS = 5.0e-3
TOL_REL = 0.0  # bf16 output of scaled d: rel err covered by TOL_ABS margin

_program_cache = {}
last_results = None  # BassKernelResults of the most recent device run


def build_program(nslots: int):
    """One SPMD program: R = nslots*128 rows, one expert, one wd quarter."""
    R = nslots * 128
    DR = mybir.MatmulPerfMode.DoubleRow
    nc = bacc.Bacc(None, target_bir_lowering=False)

    xT = nc.dram_tensor("xT", [COND, R], BF16, kind="ExternalInput")
    w0 = nc.dram_tensor("w0", [COND, 256], BF16, kind="ExternalInput")
    w1 = nc.dram_tensor("w1", [256, 512], BF16, kind="ExternalInput")
    w2 = nc.dram_tensor("w2", [512, 1024], BF16, kind="ExternalInput")
    b0 = nc.dram_tensor("b0", [256], F32, kind="ExternalInput")
    b1 = nc.dram_tensor("b1", [512], F32, kind="ExternalInput")
    b2s = nc.dram_tensor("b2s", [1024], F32, kind="ExternalInput")  # BETA*b2
    wdq = nc.dram_tensor("wdq", [1024, QP], FP8, kind="ExternalInput")  # ALPHA*wd
    dq = nc.dram_tensor("dq", [R, QP], BF16, kind="ExternalOutput")  # SCALE*d

    relu = mybir.ActivationFunctionType.Relu

    with tile.TileContext(nc) as tc:
        with (
            tc.tile_pool(name="const", bufs=1) as const,
            tc.tile_pool(name="hpool", bufs=1) as hpool,
            tc.tile_pool(name="wdpool", bufs=3) as wdpool,
            tc.tile_pool(name="opool", bufs=3) as opool,
            tc.tile_pool(name="psA", bufs=2, space="PSUM") as psA,
            tc.tile_pool(name="psB", bufs=4, space="PSUM") as psB,
        ):
            # ---- load x and small weights ----
            xt = const.tile([COND, R], BF16, name="xt")
            nc.sync.dma_start(xt[:], xT[:])

            w0t = const.tile([COND, 256], BF16, name="w0t")
            nc.scalar.dma_start(w0t[:], w0[:])
            w1t = [const.tile([128, 512], BF16, name=f"w1t{k}") for k in range(2)]
            for k in range(2):
                nc.scalar.dma_start(w1t[k][:], w1[k * 128 : (k + 1) * 128, :])
            w2t = [const.tile([128, 1024], BF16, name=f"w2t{k}") for k in range(4)]
            for k in range(4):
                nc.scalar.dma_start(w2t[k][:], w2[k * 128 : (k + 1) * 128, :])

            def bias_tiles(bsrc, dout, tag):
                ts = []
                for m in range(dout // 128):
                    t = const.tile([128, 1], F32, name=f"{tag}{m}")
                    nc.scalar.dma_start(t[:], bsrc[m * 128 : (m + 1) * 128, None])
                    ts.append(t)
                return ts

            b0t = bias_tiles(b0, 256, "b0t")
            b1t = bias_tiles(b1, 512, "b1t")
            b2t = bias_tiles(b2s, 1024, "b2t")

            # ---- small MLP, transposed layout: h[dout partitions, R free] ----
            h1 = [hpool.tile([128, R], BF16, name=f"h1_{m}") for m in range(2)]
            for m in range(2):
                pt = psA.tile([128, R], F32, name="psmall")
                nc.tensor.matmul(
                    pt[:], w0t[:, m * 128 : (m + 1) * 128], xt[:], start=True, stop=True
                )
                nc.scalar.activation(h1[m][:], pt[:], relu, bias=b0t[m][:])

            h2 = [hpool.tile([128, R], BF16, name=f"h2_{m}") for m in range(4)]
            for m in range(4):
                pt = psA.tile([128, R], F32, name="psmall")
                for k in range(2):
                    nc.tensor.matmul(
                        pt[:],
                        w1t[k][:, m * 128 : (m + 1) * 128],
                        h1[k][:],
                        start=(k == 0),
                        stop=(k == 1),
                    )
                nc.scalar.activation(h2[m][:], pt[:], relu, bias=b1t[m][:])

            # h3 in fp8, interleavable layout [128, ko=8, R]; h3 = BETA*relu(.)
            h3t = hpool.tile([128, 8, R], FP8, name="h3t")
            for m in range(8):
                pt = psA.tile([128, R], F32, name="psmall")
                for k in range(4):
                    nc.tensor.matmul(
                        pt[:],
                        w2t[k][:, m * 128 : (m + 1) * 128],
                        h2[k][:],
                        start=(k == 0),
                        stop=(k == 3),
                    )
                nc.scalar.activation(
                    h3t[:, m, :], pt[:], relu, bias=b2t[m][:], scale=BETA
                )

            # ---- big layer: dq[r, c] = SCALE * h3.T @ wd, DoubleRow fp8 ----
            wdq_t = wdq.rearrange("(ko p) n -> p ko n", p=128)  # [128, 8, QP]
            dq_t = dq.rearrange("(s p) c -> p s c", p=128)  # [128, nslots, QP]
            for wb in range(QP // 1024):
                wdt = wdpool.tile([128, 8, 1024], FP8, name="wdt")
                nc.sync.dma_start(wdt[:], wdq_t[:, :, wb * 1024 : (wb + 1) * 1024])
                ot = opool.tile([128, nslots, 1024], BF16, name="ot")
                for half in range(2):
                    csl = slice(half * 512, (half + 1) * 512)
                    for slot in range(nslots):
                        pt = psB.tile([128, 512], F32, name="pbig")
                        for j in range(4):
                            nc.tensor.matmul(
                                pt[:],
                                h3t[:, 2 * j : 2 * j + 2, slot * 128 : (slot + 1) * 128],
                                wdt[:, 2 * j : 2 * j + 2, csl],
                                start=(j == 0),
                                stop=(j == 3),
                                perf_mode=DR,
                            )
                        if slot % 2 == 0:
                            nc.scalar.copy(ot[:, slot, csl], pt[:])
                        else:
                            nc.vector.tensor_copy(ot[:, slot, csl], pt[:])
                nc.scalar.dma_start(dq_t[:, :, wb * 1024 : (wb + 1) * 1024], ot[:])
    nc.compile()
    return nc


def _ensure_ntff_hook():
    """Provide antenv.axon_hooks (absent in this image) so trace=True works."""
    import sys
    import types

    try:
        from antenv.axon_hooks import get_axon_ntff_profile_hook  # noqa: F401

        return
    except ImportError:
        pass
    try:
        import antenv
        from trn_agent_boot.trn_boot import _ntff_profile_via_ctypes

        hook = _ntff_profile_via_ctypes("/opt/axon/libaxon_pjrt.so")
        mod = types.ModuleType("antenv.axon_hooks")
        mod._hook = hook
        mod.set_axon_ntff_profile_hook = lambda h: setattr(mod, "_hook", h)
        mod.get_axon_ntff_profile_hook = lambda: mod._hook
        sys.modules["antenv.axon_hooks"] = mod
        antenv.axon_hooks = mod
    except Exception:
        pass


def _exact_h3(x, ws, bs):
    h = x.astype(np.float64)
    for i in range(3):
        h = np.maximum(h @ ws[i].astype(np.float64) + bs[i].astype(np.float64), 0)
    return h


def kernel(**inputs) -> np.ndarray:
    global last_results
    x = np.ascontiguousarray(inputs["x"], dtype=np.float32)
    gumbel = np.ascontiguousarray(inputs["gumbel"], dtype=np.float32)
    bw = [np.asarray(inputs[f"bw{i}"], dtype=np.float32) for i in range(4)]
    bb = [np.asarray(inputs[f"bb{i}"], dtype=np.float32) for i in range(4)]
    sw = [np.asarray(inputs[f"sw{i}"], dtype=np.float32) for i in range(4)]
    sb = [np.asarray(inputs[f"sb{i}"], dtype=np.float32) for i in range(4)]

    mask_big = x[:, 0] > 0.0
    b = int(mask_big.sum())
    # stable sort: big rows first, original order within groups
    perm = np.argsort(~mask_big, kind="stable")
    x_sorted = x[perm]

    def wd_of(w3):
        wd = w3[:, 0::2] - w3[:, 1::2]
        # pad each 8160-col quarter independently to 8192 cols
        wdp = np.zeros((1024, QP * 4), dtype=np.float32)
        for q in range(4):
            wdp[:, q * QP : q * QP + QCOLS] = wd[:, q * QCOLS : (q + 1) * QCOLS]
        wdp *= ALPHA
        np.clip(wdp, -240.0, 240.0, out=wdp)
        return wdp.astype(NP_FP8)

    wd8 = {"big": wd_of(bw[3]), "small": wd_of(sw[3])}
    wd_f32 = {
        "big": bw[3][:, 0::2] - bw[3][:, 1::2],
        "small": sw[3][:, 0::2] - sw[3][:, 1::2],
    }
    bdd = {"big": bb[3][0::2] - bb[3][1::2], "small": sb[3][0::2] - sb[3][1::2]}
    small_w = {"big": bw[:3], "small": sw[:3]}
    small_b = {"big": bb[:3], "small": sb[:3]}

    # chunk -> expert assignment over sorted rows
    bigchunks = [c for c in range(4) if 128 * c < b]
    smallchunks = [c for c in range(4) if 128 * (c + 1) > b]
    if b == 0:
        groups = [("small", [0, 1]), ("small", [2, 3])]
    elif b == B:
        groups = [("big", [0, 1]), ("big", [2, 3])]
    else:
        groups = [("big", bigchunks), ("small", smallchunks)]
    nslots = max(len(g[1]) for g in groups)
    slots = []
    for exp, chunks in groups:
        padded = list(chunks) + [chunks[-1]] * (nslots - len(chunks))
        slots.append((exp, padded))

    if nslots not in _program_cache:
        _program_cache[nslots] = build_program(nslots)
    nc = _program_cache[nslots]

    in_maps = []
    for g, (exp, chunks) in enumerate(slots):
        rows = np.concatenate([np.arange(128 * c, 128 * (c + 1)) for c in chunks])
        xT_g = np.ascontiguousarray(x_sorted[rows].T).astype(NP_BF16)
        for q in range(4):
            qsl = slice(q * QP, (q + 1) * QP)
            in_maps.append(
                {
                    "xT": xT_g,
                    "w0": small_w[exp][0].astype(NP_BF16),
                    "w1": small_w[exp][1].astype(NP_BF16),
                    "w2": small_w[exp][2].astype(NP_BF16),
                    "b0": small_b[exp][0],
                    "b1": small_b[exp][1],
                    "b2s": (small_b[exp][2] * BETA).astype(np.float32),
                    "wdq": np.ascontiguousarray(wd8[exp][:, qsl]),
                }
            )

    trace = bool(int(os.environ.get("CC_KERNEL_TRACE", "0")))
    if trace:
        _ensure_ntff_hook()
    try:
        res = run_bass_kernel_spmd(
            nc,
            in_maps,
            core_ids=list(range(NCORES)),
            trace=trace,
            trace_cores=list(range(NCORES)) if trace else None,
        )
    except Exception:
        if not trace:
            raise
        res = run_bass_kernel_spmd(nc, in_maps, core_ids=list(range(NCORES)))
    last_results = res

    # ---- assemble d (unscaled) in sorted row order ----
    d_sorted = np.empty((B, E), dtype=np.float32)
    for g, (exp, chunks) in enumerate(slots):
        isbig = exp == "big"
        for s, c in enumerate(chunks):
            r0, r1 = 128 * c, 128 * (c + 1)
            if 0 < b < B:
                sel = (np.arange(r0, r1) < b) == isbig
            else:
                sel = np.ones(128, dtype=bool)
            if not sel.any():
                continue
            for q in range(4):
                shard = res.results[g * 4 + q]["dq"]
                d_sorted[r0:r1, q * QCOLS : (q + 1) * QCOLS][sel] = (
                    shard[s * 128 : (s + 1) * 128, :QCOLS][sel].astype(np.float32)
                    / SCALE
                )

    # unsort rows
    d_full = np.empty_like(d_sorted)
    d_full[perm] = d_sorted
    global last_d_full
    last_d_full = d_full

    # exact gd and per-row bdd; margins
    bdd_sel = np.where(mask_big[:, None], bdd["big"][None, :], bdd["small"][None, :])
    gd = gumbel[:, :, 1].astype(np.float32) - gumbel[:, :, 0].astype(np.float32)
    t_full = d_full + bdd_sel - gd
    dec_full = t_full >= 0.0

    # ---- exact patch of near-tie edges ----
    thr = TOL_ABS + TOL_REL * np.abs(d_full)
    near_r, near_c = np.nonzero(np.abs(t_full) < thr)
    if near_r.size:
        gde = (
            gumbel[near_r, near_c, 1].astype(np.float64)
            - gumbel[near_r, near_c, 0].astype(np.float64)
        )
        for exp, msk in (("big", mask_big), ("small", ~mask_big)):
            selp = msk[near_r]
            if not selp.any():
                continue
            r, c = near_r[selp], near_c[selp]
            ws = small_w[exp]
            bs = small_b[exp]
            h3e = _exact_h3(x, ws, bs)  # [B, 1024] float64
            d = np.einsum("ij,ji->i", h3e[r], wd_f32[exp][:, c].astype(np.float64))
            m = d + bdd[exp][c] - gde[selp]
            dec_full[r, c] = m >= 0
    dec_full = dec_full.astype(np.float32)

    # ---- scatter to symmetric adjacency ----
    iu, ju = np.triu_indices(N_NODES, k=1)
    flat_idx = iu * N_NODES + ju
    out = np.zeros((B, N_NODES * N_NODES), dtype=np.float32)
    out[:, flat_idx] = dec_full
    out = out.reshape(B, N_NODES, N_NODES)
    out = out + np.swapaxes(out, 1, 2)
    return out


# revision 8
# speedup vs baseline: 1.2294x; 1.2294x over previous
"""Trainium2 Bass kernel for nn_MixMLP (moe_routing) — fp8 DoubleRow, v2.

Strategy:
  - Output is binary: y_hard + y_soft - stop_grad(y_soft) == y_hard numerically,
    so each edge decision is  (logit0 - logit1) + (gum0 - gum1) >= 0.
  - Only the DIFFERENCE of adjacent final-layer columns matters:
        d = h3 @ wd,  wd = w3[:, 0::2] - w3[:, 1::2]   (1024 x 32640)
    decision = (d + bdd - gd) >= 0,  gd = gum1 - gum0, bdd = b3[0::2]-b3[1::2].
  - Rows are routed to one expert by mask = x[:,0] > 0. Host sorts rows so
    row-chunks of 128 are single-expert; 8 cores = 2 row-groups (one per
    expert) x 4 column-quarters of wd.
  - Device computes ONLY d (scaled): small MLP in bf16, h3 emitted as
    fp8e4m3 (x BETA), wd pre-scaled by ALPHA into fp8e4m3; big matmul in
    DoubleRow mode (256-contraction, 2x fp8 rate). Output d in fp8
    (x SCALE_OUT). gumbel never touches the device.
  - v2 perf: inputs coalesced into 5 DMAs (ACT queue was serializing on
    ~22 descriptor-gens); all wd blocks prefetched upfront; output DMAs on
    the idle GpSimd (SWDGE) queue.
  - Host: dec = (t >= 0), t = d + bdd - gd; near-ties |t| < 5e-3+0.05|d|
    recomputed exactly in float64, then scattered into the symmetric
    adjacency. Result is exact regardless of device matmul precision.
"""

import os
import numpy as np
import ml_dtypes

import concourse.bass as bass
import concourse.mybir as mybir
import concourse.tile as tile
from concourse import bacc
from concourse.bass_utils import run_bass_kernel_spmd

B = 512
COND = 64
N_NODES = 256
E = 32640  # upper-tri edges
NCORES = 8
QCOLS = E // 4  # 8160 columns of wd per core
QP = 8192  # padded to multiple of 1024
ARCH = [256, 512, 1024]

ALPHA = 512.0  # wd scale into fp8
BETA = 32.0  # h3 scale into fp8
SCALE = ALPHA * BETA  # psum holds SCALE*d
SCALE_OUT = 512.0  # output tensor holds SCALE_OUT*d

F32 = mybir.dt.float32
BF16 = mybir.dt.bfloat16
FP8 = mybir.dt.float8e4
NP_FP8 = ml_dtypes.float8_e4m3
NP_BF16 = ml_dtypes.bfloat16

# |t| < TOL_ABS + TOL_REL*|d| edges are recomputed exactly on host
TOL_ABS = 5.0e-3
TOL_REL = 0.05  # covers fp8 output quantization of d

_program_cache = {}
last_results = None  # BassKernelResults of the most recent device run


def build_program(nslots: int):
    """One SPMD program: R = nslots*128 rows, one expert, one wd quarter."""
    R = nslots * 128
    DR = mybir.MatmulPerfMode.DoubleRow
    nc = bacc.Bacc(None, target_bir_lowering=False)

    xT = nc.dram_tensor("xT", [COND, R], BF16, kind="ExternalInput")
    w0 = nc.dram_tensor("w0", [COND, 256], BF16, kind="ExternalInput")
    w1 = nc.dram_tensor("w1", [256, 512], BF16, kind="ExternalInput")
    w2 = nc.dram_tensor("w2", [512, 1024], BF16, kind="ExternalInput")
    # packed biases [128, 14] f32: b0 as 2 cols, b1 as 4, BETA*b2 as 8
    bp = nc.dram_tensor("bp", [128, 14], F32, kind="ExternalInput")
    wdq = nc.dram_tensor("wdq", [1024, QP], FP8, kind="ExternalInput")  # ALPHA*wd
    dq = nc.dram_tensor("dq", [R, QP], FP8, kind="ExternalOutput")  # SCALE_OUT*d

    relu = mybir.ActivationFunctionType.Relu

    with tile.TileContext(nc) as tc:
        with (
            tc.tile_pool(name="const", bufs=1) as const,
            tc.tile_pool(name="hpool", bufs=1) as hpool,
            tc.tile_pool(name="wdpool", bufs=8) as wdpool,
            tc.tile_pool(name="opool", bufs=3) as opool,
            tc.tile_pool(name="psA", bufs=2, space="PSUM") as psA,
            tc.tile_pool(name="psB", bufs=4, space="PSUM") as psB,
        ):
            # ---- coalesced input loads: 5 descriptor-gens total ----
            xt = const.tile([COND, R], BF16, name="xt")
            nc.sync.dma_start(xt[:], xT[:])

            w0t = const.tile([COND, 256], BF16, name="w0t")
            nc.scalar.dma_start(w0t[:], w0[:])
            w1t = const.tile([128, 2, 512], BF16, name="w1t")
            nc.scalar.dma_start(w1t[:], w1.rearrange("(k p) n -> p k n", p=128))
            w2t = const.tile([128, 4, 1024], BF16, name="w2t")
            nc.scalar.dma_start(w2t[:], w2.rearrange("(k p) n -> p k n", p=128))
            bpt = const.tile([128, 14], F32, name="bpt")
            nc.scalar.dma_start(bpt[:], bp[:])

            def bias(j):
                return bpt[:, j : j + 1]

            # ---- small MLP, transposed layout: h[dout partitions, R free] ----
            h1 = [hpool.tile([128, R], BF16, name=f"h1_{m}") for m in range(2)]
            for m in range(2):
                pt = psA.tile([128, R], F32, name="psmall")
                nc.tensor.matmul(
                    pt[:], w0t[:, m * 128 : (m + 1) * 128], xt[:], start=True, stop=True
                )
                nc.scalar.activation(h1[m][:], pt[:], relu, bias=bias(m))

            h2 = [hpool.tile([128, R], BF16, name=f"h2_{m}") for m in range(4)]
            for m in range(4):
                pt = psA.tile([128, R], F32, name="psmall")
                for k in range(2):
                    nc.tensor.matmul(
                        pt[:],
                        w1t[:, k, m * 128 : (m + 1) * 128],
                        h1[k][:],
                        start=(k == 0),
                        stop=(k == 1),
                    )
                nc.scalar.activation(h2[m][:], pt[:], relu, bias=bias(2 + m))

            # h3 in fp8, interleavable layout [128, ko=8, R]; holds BETA*h3
            h3t = hpool.tile([128, 8, R], FP8, name="h3t")
            for m in range(8):
                pt = psA.tile([128, R], F32, name="psmall")
                for k in range(4):
                    nc.tensor.matmul(
                        pt[:],
                        w2t[:, k, m * 128 : (m + 1) * 128],
                        h2[k][:],
                        start=(k == 0),
                        stop=(k == 3),
                    )
                nc.scalar.activation(
                    h3t[:, m, :], pt[:], relu, bias=bias(6 + m), scale=BETA
                )

            # ---- big layer: dq[r, c] = SCALE_OUT * h3.T @ wd, DoubleRow fp8 ----
            wdq_t = wdq.rearrange("(ko p) n -> p ko n", p=128)  # [128, 8, QP]
            dq_t = dq.rearrange("(s p) c -> p s c", p=128)  # [128, nslots, QP]
            OSC = SCALE_OUT / SCALE  # psum -> out rescale (exact power of 2)
            for wb in range(QP // 1024):
                wdt = wdpool.tile([128, 8, 1024], FP8, name="wdt")
                nc.sync.dma_start(wdt[:], wdq_t[:, :, wb * 1024 : (wb + 1) * 1024])
                ot = opool.tile([128, nslots, 1024], FP8, name="ot")
                for half in range(2):
                    csl = slice(half * 512, (half + 1) * 512)
                    for slot in range(nslots):
                        pt = psB.tile([128, 512], F32, name="pbig")
                        for j in range(4):
                            nc.tensor.matmul(
                                pt[:],
                                h3t[:, 2 * j : 2 * j + 2, slot * 128 : (slot + 1) * 128],
                                wdt[:, 2 * j : 2 * j + 2, csl],
                                start=(j == 0),
                                stop=(j == 3),
                                perf_mode=DR,
                            )
                        if slot % 2 == 0:
                            nc.scalar.mul(ot[:, slot, csl], pt[:], OSC)
                        else:
                            nc.vector.tensor_scalar_mul(ot[:, slot, csl], pt[:], OSC)
                nc.gpsimd.dma_start(dq_t[:, :, wb * 1024 : (wb + 1) * 1024], ot[:])
    nc.compile()
    return nc


def _ensure_ntff_hook():
    """Provide antenv.axon_hooks (absent in this image) so trace=True works."""
    import sys
    import types

    try:
        from antenv.axon_hooks import get_axon_ntff_profile_hook  # noqa: F401

        return
    except ImportError:
        pass
    try:
        import antenv
        from trn_agent_boot.trn_boot import _ntff_profile_via_ctypes

        hook = _ntff_profile_via_ctypes("/opt/axon/libaxon_pjrt.so")
        mod = types.ModuleType("antenv.axon_hooks")
        mod._hook = hook
        mod.set_axon_ntff_profile_hook = lambda h: setattr(mod, "_hook", h)
        mod.get_axon_ntff_profile_hook = lambda: mod._hook
        sys.modules["antenv.axon_hooks"] = mod
        antenv.axon_hooks = mod
    except Exception:
        pass


def _exact_h3(x, ws, bs):
    h = x.astype(np.float64)
    for i in range(3):
        h = np.maximum(h @ ws[i].astype(np.float64) + bs[i].astype(np.float64), 0)
    return h


def kernel(**inputs) -> np.ndarray:
    global last_results
    x = np.ascontiguousarray(inputs["x"], dtype=np.float32)
    gumbel = np.ascontiguousarray(inputs["gumbel"], dtype=np.float32)
    bw = [np.asarray(inputs[f"bw{i}"], dtype=np.float32) for i in range(4)]
    bb = [np.asarray(inputs[f"bb{i}"], dtype=np.float32) for i in range(4)]
    sw = [np.asarray(inputs[f"sw{i}"], dtype=np.float32) for i in range(4)]
    sb = [np.asarray(inputs[f"sb{i}"], dtype=np.float32) for i in range(4)]

    mask_big = x[:, 0] > 0.0
    b = int(mask_big.sum())
    # stable sort: big rows first, original order within groups
    perm = np.argsort(~mask_big, kind="stable")
    x_sorted = x[perm]

    def wd_of(w3):
        wd = w3[:, 0::2] - w3[:, 1::2]
        # pad each 8160-col quarter independently to 8192 cols
        wdp = np.zeros((1024, QP * 4), dtype=np.float32)
        for q in range(4):
            wdp[:, q * QP : q * QP + QCOLS] = wd[:, q * QCOLS : (q + 1) * QCOLS]
        wdp *= ALPHA
        np.clip(wdp, -240.0, 240.0, out=wdp)
        return wdp.astype(NP_FP8)

    wd8 = {"big": wd_of(bw[3]), "small": wd_of(sw[3])}
    wd_f32 = {
        "big": bw[3][:, 0::2] - bw[3][:, 1::2],
        "small": sw[3][:, 0::2] - sw[3][:, 1::2],
    }
    bdd = {"big": bb[3][0::2] - bb[3][1::2], "small": sb[3][0::2] - sb[3][1::2]}
    small_w = {"big": bw[:3], "small": sw[:3]}
    small_b = {"big": bb[:3], "small": sb[:3]}

    def bias_pack(bs):
        bpk = np.empty((128, 14), dtype=np.float32)
        bpk[:, 0:2] = bs[0].reshape(2, 128).T
        bpk[:, 2:6] = bs[1].reshape(4, 128).T
        bpk[:, 6:14] = (BETA * bs[2]).reshape(8, 128).T
        return bpk

    # chunk -> expert assignment over sorted rows
    bigchunks = [c for c in range(4) if 128 * c < b]
    smallchunks = [c for c in range(4) if 128 * (c + 1) > b]
    if b == 0:
        groups = [("small", [0, 1]), ("small", [2, 3])]
    elif b == B:
        groups = [("big", [0, 1]), ("big", [2, 3])]
    else:
        groups = [("big", bigchunks), ("small", smallchunks)]
    nslots = max(len(g[1]) for g in groups)
    slots = []
    for exp, chunks in groups:
        padded = list(chunks) + [chunks[-1]] * (nslots - len(chunks))
        slots.append((exp, padded))

    if nslots not in _program_cache:
        _program_cache[nslots] = build_program(nslots)
    nc = _program_cache[nslots]

    in_maps = []
    for g, (exp, chunks) in enumerate(slots):
        rows = np.concatenate([np.arange(128 * c, 128 * (c + 1)) for c in chunks])
        xT_g = np.ascontiguousarray(x_sorted[rows].T).astype(NP_BF16)
        bpk = bias_pack(small_b[exp])
        for q in range(4):
            qsl = slice(q * QP, (q + 1) * QP)
            in_maps.append(
                {
                    "xT": xT_g,
                    "w0": small_w[exp][0].astype(NP_BF16),
                    "w1": small_w[exp][1].astype(NP_BF16),
                    "w2": small_w[exp][2].astype(NP_BF16),
                    "bp": bpk,
                    "wdq": np.ascontiguousarray(wd8[exp][:, qsl]),
                }
            )

    trace = bool(int(os.environ.get("CC_KERNEL_TRACE", "0")))
    if trace:
        _ensure_ntff_hook()
    try:
        res = run_bass_kernel_spmd(
            nc,
            in_maps,
            core_ids=list(range(NCORES)),
            trace=trace,
            trace_cores=list(range(NCORES)) if trace else None,
        )
    except Exception:
        if not trace:
            raise
        res = run_bass_kernel_spmd(nc, in_maps, core_ids=list(range(NCORES)))
    last_results = res

    # ---- assemble d (unscaled) in sorted row order ----
    d_sorted = np.empty((B, E), dtype=np.float32)
    for g, (exp, chunks) in enumerate(slots):
        isbig = exp == "big"
        for s, c in enumerate(chunks):
            r0, r1 = 128 * c, 128 * (c + 1)
            if 0 < b < B:
                sel = (np.arange(r0, r1) < b) == isbig
            else:
                sel = np.ones(128, dtype=bool)
            if not sel.any():
                continue
            for q in range(4):
                shard = res.results[g * 4 + q]["dq"]
                d_sorted[r0:r1, q * QCOLS : (q + 1) * QCOLS][sel] = (
                    shard[s * 128 : (s + 1) * 128, :QCOLS][sel].astype(np.float32)
                    / SCALE_OUT
                )

    # unsort rows
    d_full = np.empty_like(d_sorted)
    d_full[perm] = d_sorted
    global last_d_full
    last_d_full = d_full

    # exact gd and per-row bdd; margins
    bdd_sel = np.where(mask_big[:, None], bdd["big"][None, :], bdd["small"][None, :])
    gd = gumbel[:, :, 1].astype(np.float32) - gumbel[:, :, 0].astype(np.float32)
    t_full = d_full + bdd_sel - gd
    dec_full = t_full >= 0.0

    # ---- exact patch of near-tie edges ----
    thr = TOL_ABS + TOL_REL * np.abs(d_full)
    near_r, near_c = np.nonzero(np.abs(t_full) < thr)
    if near_r.size:
        gde = (
            gumbel[near_r, near_c, 1].astype(np.float64)
            - gumbel[near_r, near_c, 0].astype(np.float64)
        )
        for exp, msk in (("big", mask_big), ("small", ~mask_big)):
            selp = msk[near_r]
            if not selp.any():
                continue
            r, c = near_r[selp], near_c[selp]
            ws = small_w[exp]
            bs = small_b[exp]
            h3e = _exact_h3(x, ws, bs)  # [B, 1024] float64
            d = np.einsum("ij,ji->i", h3e[r], wd_f32[exp][:, c].astype(np.float64))
            m = d + bdd[exp][c] - gde[selp]
            dec_full[r, c] = m >= 0
    dec_full = dec_full.astype(np.float32)

    # ---- scatter to symmetric adjacency ----
    iu, ju = np.triu_indices(N_NODES, k=1)
    flat_idx = iu * N_NODES + ju
    out = np.zeros((B, N_NODES * N_NODES), dtype=np.float32)
    out[:, flat_idx] = dec_full
    out = out.reshape(B, N_NODES, N_NODES)
    out = out + np.swapaxes(out, 1, 2)
    return out
